# revision 1
# baseline (speedup 1.0000x reference)
"""DotProductDistributionHead kernel for Trainium2 (Bass/Tile), 8-core data-parallel.

Computation (per reference):
    h = gelu(x @ W_mu + b_mu)            # (B, D) with erf gelu
    logits[b, n] = h[b] . emb_table[candidates[b, n]] + mu_bias[candidates[b, n]]

Sharding: x/candidates split along batch across 8 cores; W_mu / b_mu /
emb_table replicated per core (each core's copy lands in its own HBM
stack, so gather bandwidth scales with cores).

Gather strategy: the batch SWDGE gather (`dma_gather`) takes signed int16
row indices, so the 100K-row table is covered in 4 passes of <=32768 rows
(in_ap offset per pass). On the host, each 128-row batch block's candidates
are split by pass and packed into per-partition columns (partition p =
batch row within the block), padded with index 0 up to the per-(block,pass)
column maximum T. Each gathered row lands on the partition of its batch row,
so the dot against h is a broadcast multiply + segmented reduce per
partition; the host reassembles logits[b, n] from the packed columns.

mu_bias is all-zeros per the problem spec; a host-side fallback adds it if
a nonzero vector is ever passed.
"""

import os

import numpy as np

import concourse.bacc as bacc
import concourse.bass as bass
import concourse.tile as tile
from concourse import mybir
from concourse.bass_utils import run_bass_kernel_spmd

B, N, D, V = 4096, 200, 128, 100000
NCORES = 8
B_LOC = B // NCORES          # 512 batch rows per core
NBLK = B_LOC // 128          # 4 blocks of 128 rows
PASS_SIZE = 32768
NPASS = (V + PASS_SIZE - 1) // PASS_SIZE   # 4
CHUNK_T = 48                 # gather columns per device chunk

USE_SCAN = False             # custom DVE fused mul+scan (faster path)
TRACE = False
LAST_RESULTS = None
ACT_FUNC = "Gelu"            # overridden in sim-debug (CoreSim lacks Gelu)
REPEATS = 1                  # bench-only: emit the main loop R times (slope timing)

_f32 = mybir.dt.float32
_i16 = mybir.dt.int16

CONST_COLS = D + B_LOC + D   # [W | xT | b_mu replicated]

_program_cache = {}


def _chunks(total):
    out = []
    t0 = 0
    while t0 < total:
        out.append((t0, min(CHUNK_T, total - t0)))
        t0 += CHUNK_T
    return out


def _kernel_body(tc, consts, gidx, emb, out, t_table, out_cols):
    """t_table: T[c][k] column counts; out_cols: per-(c,k) output column offset."""
    nc = tc.nc
    gelu = getattr(mybir.ActivationFunctionType, ACT_FUNC)
    total_words = sum(t * 8 for row in t_table for t in row)

    with (
        tc.tile_pool(name="const", bufs=1) as cpool,
        tc.tile_pool(name="psum", bufs=2, space="PSUM") as ppool,
        tc.tile_pool(name="outs", bufs=2) as outpool,
        tc.tile_pool(name="gather", bufs=4) as gpool,
        tc.tile_pool(name="scratch", bufs=2) as spool,
    ):
        c_sb = cpool.tile([128, CONST_COLS], _f32)
        nc.sync.dma_start(c_sb[:], consts[:, :])
        W_sb = c_sb[:, 0:D]
        xT_sb = c_sb[:, D : D + B_LOC]
        bias_sb = c_sb[:, D + B_LOC : D + B_LOC + D]

        gidx_sb = cpool.tile([128, total_words], _i16)
        nc.sync.dma_start(gidx_sb[:], gidx[:, :])

        # h[b, d] for all 512 local rows: block c lives at h_sb[:, c*D:(c+1)*D]
        h_sb = cpool.tile([128, NBLK * D], _f32)
        for c in range(NBLK):
            ps = ppool.tile([128, D], _f32)
            nc.tensor.matmul(
                out=ps[:], lhsT=xT_sb[:, c * 128 : (c + 1) * 128], rhs=W_sb,
                start=True, stop=True,
            )
            nc.vector.tensor_tensor(
                out=ps[:], in0=ps[:], in1=bias_sb, op=mybir.AluOpType.add
            )
            nc.scalar.activation(out=h_sb[:, c * D : (c + 1) * D], in_=ps[:], func=gelu)

        lim_blk = int(os.environ.get("KERNEL_NBLK", NBLK))
        lim_pass = int(os.environ.get("KERNEL_NPASS", NPASS))
        lim_chunk = int(os.environ.get("KERNEL_NCHUNK", 10**6))
        qrr = 0
        for _rep in range(REPEATS):
          word_off = 0
          for c in range(NBLK):
            h_blk = h_sb[:, c * D : (c + 1) * D]
            t_tot = sum(t_table[c])
            logits_sb = outpool.tile([128, max(t_tot, 1)], _f32)
            if (lim_blk < NBLK or lim_pass < NPASS or os.environ.get("KERNEL_SKIP_COMPUTE")) and not os.environ.get("KERNEL_SKIP_OUT"):
                nc.vector.memset(logits_sb[:], 0.0)
            col = 0
            for k in range(NPASS):
                T = t_table[c][k]
                if T == 0:
                    continue
                if c >= lim_blk or k >= lim_pass:
                    word_off += T * 8
                    col += T
                    continue
                emb_k = emb[k * PASS_SIZE :, :]
                for ci, (t0, tc_len) in enumerate(_chunks(T)):
                    if ci >= lim_chunk:
                        continue
                    num = 128 * tc_len
                    G = gpool.tile([128, CHUNK_T * D], _f32, tag="gtile")
                    if not os.environ.get("KERNEL_SKIP_GATHER"):
                        nc.gpsimd.dma_gather(
                            out_ap=G[:, : tc_len * D].rearrange("p (t d) -> p t d", d=D),
                            in_ap=emb_k,
                            idxs_ap=gidx_sb[:, word_off + t0 * 8 : word_off + (t0 + tc_len) * 8],
                            num_idxs=num,
                            num_idxs_reg=num,
                            elem_size=D,
                            single_packet=False,
                            queue_num=qrr % 4,
                        )
                        qrr += 1
                    if os.environ.get("KERNEL_SKIP_COMPUTE"):
                        continue
                    G3 = G[:, : tc_len * D].rearrange("p (t d) -> p t d", d=D)
                    h_bc = h_blk.unsqueeze(1).to_broadcast([128, tc_len, D])
                    prod = spool.tile([128, CHUNK_T * D], _f32, tag="ptile")
                    nc.vector.tensor_tensor(
                        out=prod[:, : tc_len * D].rearrange("p (t d) -> p t d", d=D),
                        in0=G3, in1=h_bc, op=mybir.AluOpType.mult,
                    )
                    nc.vector.tensor_reduce(
                        out=logits_sb[:, col + t0 : col + t0 + tc_len],
                        in_=prod[:, : tc_len * D].rearrange("p (t d) -> p t d", d=D),
                        axis=mybir.AxisListType.X,
                        op=mybir.AluOpType.add,
                    )
                word_off += T * 8
                col += T
            if not os.environ.get("KERNEL_SKIP_OUT"):
                nc.sync.dma_start(
                    out[:, out_cols[c] : out_cols[c] + t_tot], logits_sb[:, :t_tot]
                )


def _build_program(t_table, out_cols, total_out_cols):
    key = (tuple(tuple(r) for r in t_table), ACT_FUNC, USE_SCAN, CHUNK_T, REPEATS)
    if key in _program_cache:
        return _program_cache[key]
    nc = bacc.Bacc(
        "TRN2",
        target_bir_lowering=False,
        debug=False,
        enable_asserts=False,
        num_devices=NCORES,
        num_swdge_queues=4,
    )
    total_words = sum(t * 8 for row in t_table for t in row)
    consts = nc.dram_tensor("consts", (128, CONST_COLS), _f32, kind="ExternalInput").ap()
    gidx = nc.dram_tensor("gidx", (128, total_words), _i16, kind="ExternalInput").ap()
    emb = nc.dram_tensor("emb", (V, D), _f32, kind="ExternalInput").ap()
    out = nc.dram_tensor("out", (128, total_out_cols), _f32, kind="ExternalOutput").ap()
    with tile.TileContext(nc) as tc:
        _kernel_body(tc, consts, gidx, emb, out, t_table, out_cols)
    nc.finalize()
    _program_cache[key] = nc
    return nc


def _pack_core(cand_local):
    """Split one core's candidates (B_LOC, N) into per-(block, pass) packed
    index columns. Returns (counts[c][k][p], idx_lists[c][k] as [128, T_ck]
    arrays unpadded-count info, assembly info)."""
    per_ck = []
    for c in range(NBLK):
        blk = cand_local[c * 128 : (c + 1) * 128]  # [128, N]
        row = []
        for k in range(NPASS):
            lo, hi = k * PASS_SIZE, min((k + 1) * PASS_SIZE, V)
            mask = (blk >= lo) & (blk < hi)
            row.append(mask)
        per_ck.append(row)
    return per_ck


def prepare(x, candidates, W_mu, b_mu, mu_bias, emb_table):
    """Host packing: returns (nc, in_maps, col_maps)."""
    x = np.asarray(x, dtype=np.float32)
    candidates = np.asarray(candidates).astype(np.int64)
    W_mu = np.ascontiguousarray(np.asarray(W_mu, dtype=np.float32))
    b_mu = np.asarray(b_mu, dtype=np.float32)
    mu_bias = np.asarray(mu_bias, dtype=np.float32)
    emb = np.ascontiguousarray(np.asarray(emb_table, dtype=np.float32))

    # --- host packing ------------------------------------------------------
    core_masks = []
    for core in range(NCORES):
        cl = candidates[core * B_LOC : (core + 1) * B_LOC]
        core_masks.append(_pack_core(cl))

    # common T per (block, pass) across cores (one SPMD program)
    t_table = [[0] * NPASS for _ in range(NBLK)]
    for c in range(NBLK):
        for k in range(NPASS):
            m = max(core_masks[core][c][k].sum(axis=1).max() for core in range(NCORES))
            t_table[c][k] = int(m)
    out_cols = []
    acc = 0
    for c in range(NBLK):
        out_cols.append(acc)
        acc += sum(t_table[c])
    total_out_cols = max(acc, 1)
    total_words = sum(t * 8 for row in t_table for t in row)

    # per-core packed index tiles + assembly map
    gidx_tiles = []
    col_maps = []  # per core: [B_LOC, N] -> column in out
    for core in range(NCORES):
        cl = candidates[core * B_LOC : (core + 1) * B_LOC]
        words = np.zeros((128, total_words), dtype=np.uint16)
        col_map = np.zeros((B_LOC, N), dtype=np.int64)
        woff = 0
        for c in range(NBLK):
            blk = cl[c * 128 : (c + 1) * 128]
            col = 0
            for k in range(NPASS):
                T = t_table[c][k]
                if T == 0:
                    continue
                mask = core_masks[core][c][k]
                I = np.zeros((128, T), dtype=np.uint16)
                for p in range(128):
                    ns = np.nonzero(mask[p])[0]
                    I[p, : len(ns)] = (blk[p, ns] - k * PASS_SIZE).astype(np.uint16)
                    col_map[c * 128 + p, ns] = out_cols[c] + col + np.arange(len(ns))
                # wrap: value for (p_dest=j%128, t=j//128) at word (j%16, j//16)
                vals_flat = I.T.ravel()  # j = t*128 + p
                wrapped = vals_flat.reshape(T * 8, 16).T  # [16, T*8]
                words[:, woff : woff + T * 8] = np.tile(wrapped, (8, 1))
                woff += T * 8
                col += T
        gidx_tiles.append(words.view(np.int16))
        col_maps.append(col_map)

    nc = _build_program(t_table, out_cols, total_out_cols)

    bias_tile = np.broadcast_to(b_mu.reshape(1, D), (128, D))
    in_maps = []
    for core in range(NCORES):
        sl = slice(core * B_LOC, (core + 1) * B_LOC)
        consts = np.concatenate([W_mu, x[sl].T, bias_tile], axis=1)
        in_maps.append(
            {
                "consts": np.ascontiguousarray(consts, dtype=np.float32),
                "gidx": np.ascontiguousarray(gidx_tiles[core]),
                "emb": emb,
            }
        )
    return nc, in_maps, col_maps


def assemble(results, col_maps):
    """results: per-core dicts with 'out' [128, total_cols] -> logits (B, N)."""
    ncores_run = len(results)
    logits = np.zeros((B, N), dtype=np.float32)
    for core in range(ncores_run):
        out_core = results[core]["out"]
        cm = col_maps[core]
        for c in range(NBLK):
            rows = slice(c * 128, (c + 1) * 128)
            logits[core * B_LOC + c * 128 : core * B_LOC + (c + 1) * 128] = (
                np.take_along_axis(out_core, cm[rows], axis=1)
            )
    return logits


def kernel(x, candidates, W_mu, b_mu, mu_bias, emb_table):
    global LAST_RESULTS
    candidates = np.asarray(candidates).astype(np.int64)
    mu_bias = np.asarray(mu_bias, dtype=np.float32)
    nc, in_maps, col_maps = prepare(x, candidates, W_mu, b_mu, mu_bias, emb_table)
    ncores_run = int(os.environ.get("KERNEL_CORES", NCORES))
    res = run_bass_kernel_spmd(
        nc, in_maps[:ncores_run], core_ids=list(range(ncores_run)), trace=TRACE
    )
    LAST_RESULTS = res
    logits = assemble(res.results, col_maps[:ncores_run])
    if np.any(mu_bias):
        logits = logits + mu_bias[candidates]
    return np.ascontiguousarray(logits.astype(np.float32))

